# revision 40
# baseline (speedup 1.0000x reference)
"""Multi-head self-attention (B=8, N=1024, C=768, H=12) on 8 trn2 NeuronCores.

Sharding: data-parallel over batch — core b computes batch element b end to
end; weights are replicated. No collectives.

Per-core dataflow (all matmuls on TensorE, out = lhsT.T @ rhs, contraction on
the partition dim):

  1. qkv^T for Q,K in [c', n] layout:  lhsT = Wqkv^T k-tile, rhs = x^T k-tile.
     Bias is per-partition (c' rows) -> fused into the PSUM->SBUF copy on DVE.
  2. V in token-major [n, c'] layout:  lhsT = x^T k-tile, rhs = Wqkv^T slice.
     V bias is skipped on-device: since softmax rows sum to 1, it folds into
     an adjusted proj bias  bp' = b_proj + W_proj @ b_qkv[V]  (host-computed).
     V is stored per-head as [V_h | 1] (65 cols per head): the ones column
     makes the A@V matmul also produce the softmax row-sums.
  3. Per head h: S^T[m, n] = (K_h^T).T-stationary @ Q_h^T (K = d = 64).
     exp via ScalarE reading PSUM, writing SBUF (scale=1/sqrt(64) folded in;
     max-subtraction skipped — scores are O(1) in this problem so exp is
     safe, and softmax is shift-invariant so the result is identical).
  4. O_u^T[d, n] (+ row-sums s[n] in partition 64) accumulated over m-tiles
     with stationary [V_h | 1] (M = 65).
  5. recip = 1/s via the custom-DVE fast reciprocal; broadcast across 64
     partitions via GPSIMD partition_broadcast; normalization fused into the
     PSUM->SBUF move (tensor_mul), writing O^T[c, n] stacked across heads.
  6. y[n, co] = proj with O^T tiles stationary -> token-major output; proj
     bias pre-broadcast to [128, C] once (GPSIMD) and added by DVE during
     the PSUM->SBUF move.

Scheduling: the kernel is ScalarE(exp)-bound in attention and TensorE-bound
overall, so the emission order software-pipelines everything around the
in-order engine queues: each head's AV matmuls are deferred one full head
(PE never waits on the exp latency, PSUM 'o' slots never gate the next
head's scores), and all independent projection work (V, later heads' Q/K
tiles) is drained one unit per m-tile slot inside the ACT-bound attention
stream. PSUM budget (8 banks): 2x[128,1024] score tiles + 4 banks shared by
AV accumulators and filler groups.

All matmul inputs are bf16 (fp32 accumulate); measured end-to-end error vs
the fp32 reference is ~2.4e-3 scale-relative. ~198us on hardware per core.
"""

import numpy as np
import ml_dtypes

B, N, C = 8, 1024, 768
H, D = 12, 64
HD = D + 1  # per-head V block width incl. ones column
N_CORES = 8
P = 128
KT = C // P  # 6 contraction tiles
NT = N // P  # 8 token tiles

_CACHE: dict = {}


def _build(cfg: dict):
    import concourse.bass as bass
    import concourse.bacc as bacc
    import concourse.mybir as mybir
    import concourse.tile as tile

    dt = mybir.dt
    f32 = dt.float32
    dt_qkv = getattr(dt, cfg["dt_qkv"])    # x, Wqkv storage (qkv matmul ins)
    dt_qk = getattr(dt, cfg["dt_qk"])      # Q^T/K^T storage (scores matmul ins)
    dt_av = getattr(dt, cfg["dt_av"])      # E, V storage (AV matmul ins)
    dt_proj = getattr(dt, cfg["dt_proj"])  # O^T, Wproj storage (proj matmul ins)
    # matmul-issue dtypes (may be float32r views of float32 storage)
    mm_qkv = getattr(dt, cfg["mm_qkv"])
    mm_qk = getattr(dt, cfg["mm_qk"])
    mm_av = getattr(dt, cfg["mm_av"])
    mm_proj = getattr(dt, cfg["mm_proj"])

    nc = bacc.Bacc("TRN2", target_bir_lowering=False, debug=False,
                   num_devices=N_CORES)

    xT_d = nc.dram_tensor("xT", [C, N], dt_qkv, kind="ExternalInput")
    wqkvT_d = nc.dram_tensor("wqkvT", [C, 3 * C], dt_qkv, kind="ExternalInput")
    wprojT_d = nc.dram_tensor("wprojT", [C, C], dt_proj, kind="ExternalInput")
    bqk_d = nc.dram_tensor("bqk", [P, 2 * C // P], f32, kind="ExternalInput")
    bp_d = nc.dram_tensor("bp", [1, C], f32, kind="ExternalInput")
    y_d = nc.dram_tensor("y", [N, C], f32, kind="ExternalOutput")

    def mmv(ap, mmdt):
        # reinterpret a float32 AP as float32r for fast matmul issue
        return ap.bitcast(mmdt) if mmdt != ap.dtype else ap

    with tile.TileContext(nc, pool_alloc_mode="queue") as tc:
        with (
            tc.tile_pool(name="const", bufs=1) as cpool,
            tc.tile_pool(name="et", bufs=cfg["et_bufs"]) as etpool,
            tc.tile_pool(name="work", bufs=2) as workpool,
            tc.tile_pool(name="ps_s", bufs=2, space="PSUM") as ps_s,
            tc.tile_pool(name="ps_o", bufs=4, space="PSUM") as ps_o,
        ):
            # ---- resident loads ----
            # x + small tensors on the Sync HWDGE queue; the big Wqkv on the
            # Scalar HWDGE queue so the two streams load in parallel.
            xT = [cpool.tile([P, N], dt_qkv, name=f"xT{k}", tag=f"xT{k}") for k in range(KT)]
            for k in range(KT):
                nc.sync.dma_start(xT[k][:], xT_d.ap()[k * P:(k + 1) * P, :])
            # Q/K weight columns on the Scalar queue, V columns on the GpSimd
            # (SWDGE) queue — three DMA streams load in parallel
            wq = [cpool.tile([P, 3 * C], dt_qkv, name=f"wq{k}", tag=f"wq{k}") for k in range(KT)]
            for k in range(KT):
                nc.scalar.dma_start(wq[k][:, 0:2 * C],
                                    wqkvT_d.ap()[k * P:(k + 1) * P, 0:2 * C])
            for k in range(KT):
                nc.gpsimd.dma_start(wq[k][:, 2 * C:3 * C],
                                    wqkvT_d.ap()[k * P:(k + 1) * P, 2 * C:3 * C])
            bqk = cpool.tile([P, 2 * C // P], f32, name="bqk", tag="bqk")
            nc.sync.dma_start(bqk[:], bqk_d.ap())
            bp = cpool.tile([1, C], f32, name="bp", tag="bp")
            nc.sync.dma_start(bp[:], bp_d.ap())
            bp_b = cpool.tile([P, C], f32, name="bp_b", tag="bp_b")
            nc.gpsimd.partition_broadcast(bp_b[:], bp[:])
            wp = [cpool.tile([P, C], dt_proj, name=f"wp{k}", tag=f"wp{k}") for k in range(KT)]
            for k in range(KT):
                nc.sync.dma_start(wp[k][:], wprojT_d.ap()[k * P:(k + 1) * P, :])

            # ---- phase 1: Q^T, K^T in [c', n] tiles ----
            # Only the first two head-pairs' tiles are emitted up front; the
            # rest are interleaved into the attention stream as PE filler
            # (attention is ACT/exp-bound, PE has idle slots), two pairs
            # ahead of their consumer so the scheduler has slack.
            qkT = [cpool.tile([P, N], dt_qk, name=f"qkT{t}", tag=f"qkT{t}")
                   for t in range(2 * C // P)]

            def qk_group(t, g, pool, tag):
                pm = pool.tile([P, 512], f32, name="mm", tag=tag)
                for k in range(KT):
                    nc.tensor.matmul(
                        pm[:],
                        mmv(wq[k][:, t * P:(t + 1) * P], mm_qkv),
                        mmv(xT[k][:, g * 512:(g + 1) * 512], mm_qkv),
                        start=(k == 0), stop=(k == KT - 1),
                    )
                nc.vector.tensor_scalar_add(
                    qkT[t][:, g * 512:(g + 1) * 512], pm[:],
                    bqk[:, t:t + 1])

            NQT = 2 * C // P  # 12 q/k tiles; pair p uses tiles p and 6+p

            # pair 0 up front — unblocks attention immediately
            for t in (0, NQT // 2):
                for g in range(2):
                    qk_group(t, g, ps_o, "o")

            # ---- phase 2: V token-major with ones columns; emitted as
            # filler units inside h0's slots (h0 has no AV work yet)
            v = [cpool.tile([P, H * HD], dt_av, name=f"v{nt}", tag=f"v{nt}")
                 for nt in range(NT)]

            def v_unit(nt):
                nc.vector.memset(
                    v[nt][:].rearrange("p (h d) -> p h d", d=HD)[:, :, D:HD],
                    1.0)
                dst = v[nt][:].rearrange("p (h d) -> p h d", d=HD)
                for off, width in ((0, 512), (512, 256)):
                    pm = ps_o.tile([P, 512], f32, name="mm", tag="o")
                    for k in range(KT):
                        nc.tensor.matmul(
                            pm[:, 0:width],
                            mmv(xT[k][:, nt * P:(nt + 1) * P], mm_qkv),
                            mmv(wq[k][:, 2 * C + off:2 * C + off + width],
                                mm_qkv),
                            start=(k == 0), stop=(k == KT - 1),
                        )
                    nh, h0_ = width // D, off // D
                    src = pm[:, 0:width].rearrange("p (h d) -> p h d", d=D)
                    nc.vector.tensor_copy(dst[:, h0_:h0_ + nh, 0:D], src[:])

            ouT = [cpool.tile([P, N], dt_proj, name=f"ouT{j}", tag=f"ouT{j}")
                   for j in range(KT)]
            ysb = [workpool.tile([P, C], f32, name=f"ysb{nt}", tag=f"ysb{nt}",
                                 bufs=1) for nt in range(NT)]

            def proj_unit(nt, g, ks, first):
                # one off-group partial proj for y[nt]; accumulates into the
                # persistent SBUF ysb tile (bias folded into the first slice)
                off, width = (0, 512) if g == 0 else (512, 256)
                pm = ps_o.tile([P, 512], f32, name="mm", tag="o")
                for i, k in enumerate(ks):
                    nc.tensor.matmul(
                        pm[:, 0:width],
                        mmv(ouT[k][:, nt * P:(nt + 1) * P], mm_proj),
                        mmv(wp[k][:, off:off + width], mm_proj),
                        start=(i == 0), stop=(i == len(ks) - 1),
                    )
                sl = slice(off, off + width)
                other = bp_b[:, sl] if first else ysb[nt][:, sl]
                nc.vector.tensor_add(ysb[nt][:, sl], pm[:, 0:width], other)

            # Filler schedule, one unit per (head, m-tile) slot. V(nt) pops
            # at h0's slot nt (its consumer av(h0, nt) runs a full head
            # later). Pair p's q/k groups pop at head p, slots 4-7 (away
            # from head-boundary PSUM pressure), ready before head 2p.
            # Proj k=0..2 partials pop at heads 7-10 slots 3-6, inside the
            # steady-state PE slack (ouT[2] is final once head 5's deferred
            # normalize runs, at the end of head 6's emission).
            slot_fill: dict[tuple[int, int], object] = {}
            for nt in range(NT):
                slot_fill[(0, nt)] = (lambda nt=nt: v_unit(nt))
            for p in range(1, NQT // 2):
                units = [(lambda t=t, g=g: qk_group(t, g, ps_o, "o"))
                         for t in (p, NQT // 2 + p) for g in range(2)]
                for i, u in enumerate(units):
                    slot_fill[(p, 4 + i)] = u
            for u in range(2 * NT):
                slot_fill[(7 + u // 4, 3 + u % 4)] = (
                    lambda nt=u // 2, g=u % 2:
                        proj_unit(nt, g, [0, 1, 2], True))

            class HeadState:
                def __init__(self, h):
                    self.h = h
                    self.off = D * (h % 2)
                    self.ets = []
                    self.o_ps = None

            def score_exp(st, mt):
                qt = qkT[st.h // 2]
                kt = qkT[NQT // 2 + st.h // 2]
                sp = ps_s.tile([P, N], f32, name="sp", tag="s")
                for g in range(2):
                    nc.tensor.matmul(
                        sp[:, g * 512:(g + 1) * 512],
                        mmv(kt[st.off:st.off + D, mt * P:(mt + 1) * P], mm_qk),
                        mmv(qt[st.off:st.off + D,
                               g * 512:(g + 1) * 512], mm_qk),
                        start=True, stop=True,
                    )
                et = etpool.tile([P, N], dt_av, name="et", tag="et")
                nc.scalar.activation(
                    et[:], sp[:],
                    bass.mybir.ActivationFunctionType.Exp,
                    scale=float(1.0 / np.sqrt(D)))
                st.ets.append(et)

            def av(st, mt):
                if st.o_ps is None:
                    st.o_ps = [ps_o.tile([HD, 512], f32, name="o_ps", tag="o")
                               for _ in range(2)]
                for g in range(2):
                    nc.tensor.matmul(
                        st.o_ps[g][:],
                        mmv(v[mt][:, st.h * HD:(st.h + 1) * HD], mm_av),
                        mmv(st.ets[mt][:, g * 512:(g + 1) * 512], mm_av),
                        start=(mt == 0), stop=(mt == NT - 1),
                    )

            def normalize(st):
                # one independent chain per 512-half so PSUM slots free as
                # early as possible. The sum row is staged via SBUF: the
                # custom-DVE reciprocal mis-reads PSUM at base partition 64
                # on HW (sim is fine).
                s_sb = workpool.tile([1, N], f32, name="s_sb", tag="s_sb")
                r = workpool.tile([1, N], f32, name="r", tag="r")
                rb = workpool.tile([D, N], f32, name="rb", tag="rb")
                for g in range(2):
                    sl = slice(g * 512, (g + 1) * 512)
                    nc.vector.tensor_copy(s_sb[0:1, sl], st.o_ps[g][D:HD, :])
                    nc.vector.reciprocal_approx_fast(r[0:1, sl],
                                                     s_sb[0:1, sl])
                    nc.gpsimd.partition_broadcast(rb[:, sl], r[0:1, sl])
                    nc.vector.tensor_mul(
                        ouT[st.h // 2][st.off:st.off + D, sl],
                        st.o_ps[g][0:D, :], rb[:, sl])

            # Heads are software-pipelined one full head deep: head h's slots
            # run its scores/exp plus head h-1's AV matmuls, so the in-order
            # PE queue never waits on the exp latency and head h-1's PSUM
            # tail never blocks head h's scores.
            prev = None
            for h in range(H):
                st = HeadState(h)
                for mt in range(NT):
                    score_exp(st, mt)
                    if prev is not None:
                        av(prev, mt)
                    u = slot_fill.pop((h, mt), None)
                    if u is not None:
                        u()
                if prev is not None:
                    normalize(prev)
                prev = st
            for mt in range(NT):
                av(prev, mt)
            normalize(prev)

            # ---- phase 4 tail: only the last proj k-slices (heads 6..11)
            # remain; k=0..2 + bias already accumulated into ysb. Uses the
            # scores PSUM slots (idle by now) so n-tiles double-buffer.
            for nt in range(NT):
                pm = ps_s.tile([P, N], f32, name="mm", tag="s")
                for off, width in ((0, 512), (512, 256)):
                    for i, k in enumerate((KT - 3, KT - 2, KT - 1)):
                        nc.tensor.matmul(
                            pm[:, off:off + width],
                            mmv(ouT[k][:, nt * P:(nt + 1) * P], mm_proj),
                            mmv(wp[k][:, off:off + width], mm_proj),
                            start=(i == 0), stop=(i == 2),
                        )
                nc.vector.tensor_add(ysb[nt][:], pm[:, 0:C], ysb[nt][:])
                nc.sync.dma_start(y_d.ap()[nt * P:(nt + 1) * P, :], ysb[nt][:])

    nc.compile()
    return nc


DEFAULT_CFG = dict(
    dt_qkv="bfloat16", dt_qk="bfloat16", dt_av="bfloat16", dt_proj="bfloat16",
    mm_qkv="bfloat16", mm_qk="bfloat16", mm_av="bfloat16", mm_proj="bfloat16",
    et_bufs=12,
)


def _np_dt(name):
    return {"bfloat16": ml_dtypes.bfloat16, "float32": np.float32}[name]


def _host_prep(x, W_qkv, b_qkv, W_proj, b_proj, cfg):
    """Shard + lay out host-side numpy inputs per core."""
    dqkv = _np_dt(cfg["dt_qkv"])
    dproj = _np_dt(cfg["dt_proj"])
    wqkvT = np.ascontiguousarray(W_qkv.T).astype(dqkv)
    wprojT = np.ascontiguousarray(W_proj.T).astype(dproj)
    bqk = np.ascontiguousarray(
        b_qkv[:2 * C].reshape(2 * C // P, P).T).astype(np.float32)
    bp_eff = (b_proj.astype(np.float64)
              + W_proj.astype(np.float64) @ b_qkv[2 * C:].astype(np.float64))
    bp = bp_eff.astype(np.float32).reshape(1, C)
    in_maps = []
    for b in range(N_CORES):
        xT = np.ascontiguousarray(x[b].T).astype(dqkv)
        in_maps.append({"xT": xT, "wqkvT": wqkvT, "wprojT": wprojT,
                        "bqk": bqk, "bp": bp})
    return in_maps


def get_nc(cfg=None):
    cfg = dict(DEFAULT_CFG, **(cfg or {}))
    key = tuple(sorted(cfg.items()))
    if key not in _CACHE:
        _CACHE[key] = _build(cfg)
    return _CACHE[key]


def run(inputs, cfg=None, **run_kwargs):
    from concourse import bass_utils

    cfg = dict(DEFAULT_CFG, **(cfg or {}))
    nc = get_nc(cfg)
    in_maps = _host_prep(inputs["x"], inputs["W_qkv"], inputs["b_qkv"],
                         inputs["W_proj"], inputs["b_proj"], cfg)
    res = bass_utils.run_bass_kernel_spmd(
        nc, in_maps, core_ids=list(range(N_CORES)), **run_kwargs)
    out = np.stack([res.results[b]["y"] for b in range(N_CORES)], axis=0)
    return out, res


def kernel(**inputs) -> np.ndarray:
    inputs = {k: np.asarray(v) for k, v in inputs.items()}
    out, _ = run(inputs)
    return out


# revision 41
# speedup vs baseline: 1.0222x; 1.0222x over previous
"""Multi-head self-attention (B=8, N=1024, C=768, H=12) on 8 trn2 NeuronCores.

Sharding: data-parallel over batch — core b computes batch element b end to
end; weights are replicated. No collectives.

Per-core dataflow (all matmuls on TensorE, out = lhsT.T @ rhs, contraction on
the partition dim):

  1. qkv^T for Q,K in [c', n] layout:  lhsT = Wqkv^T k-tile, rhs = x^T k-tile.
     Bias is per-partition (c' rows) -> fused into the PSUM->SBUF copy on DVE.
  2. V in token-major [n, c'] layout:  lhsT = x^T k-tile, rhs = Wqkv^T slice.
     V bias is skipped on-device: since softmax rows sum to 1, it folds into
     an adjusted proj bias  bp' = b_proj + W_proj @ b_qkv[V]  (host-computed).
     V is stored per-head as [V_h | 1] (65 cols per head): the ones column
     makes the A@V matmul also produce the softmax row-sums.
  3. Per head h: S^T[m, n] = (K_h^T).T-stationary @ Q_h^T (K = d = 64).
     exp via ScalarE reading PSUM, writing SBUF (scale=1/sqrt(64) folded in;
     max-subtraction skipped — scores are O(1) in this problem so exp is
     safe, and softmax is shift-invariant so the result is identical).
  4. O_u^T[d, n] (+ row-sums s[n] in partition 64) accumulated over m-tiles
     with stationary [V_h | 1] (M = 65).
  5. recip = 1/s via the custom-DVE fast reciprocal; broadcast across 64
     partitions via GPSIMD partition_broadcast; normalization fused into the
     PSUM->SBUF move (tensor_mul), writing O^T[c, n] stacked across heads.
  6. y[n, co] = proj with O^T tiles stationary -> token-major output; proj
     bias pre-broadcast to [128, C] once (GPSIMD) and added by DVE during
     the PSUM->SBUF move.

Scheduling: the kernel is ScalarE(exp)-bound in attention and TensorE-bound
overall, so the emission order software-pipelines everything around the
in-order engine queues: each head's AV matmuls are deferred one full head
(PE never waits on the exp latency, PSUM 'o' slots never gate the next
head's scores), and all independent projection work (V, later heads' Q/K
tiles) is drained one unit per m-tile slot inside the ACT-bound attention
stream. PSUM budget (8 banks): 2x[128,1024] score tiles + 4 banks shared by
AV accumulators and filler groups.

All matmul inputs are bf16 (fp32 accumulate); measured end-to-end error vs
the fp32 reference is ~2.4e-3 scale-relative. ~198us on hardware per core.
"""

import numpy as np
import ml_dtypes

B, N, C = 8, 1024, 768
H, D = 12, 64
HD = D + 1  # per-head V block width incl. ones column
N_CORES = 8
P = 128
KT = C // P  # 6 contraction tiles
NT = N // P  # 8 token tiles

_CACHE: dict = {}


def _build(cfg: dict):
    import concourse.bass as bass
    import concourse.bacc as bacc
    import concourse.mybir as mybir
    import concourse.tile as tile

    dt = mybir.dt
    f32 = dt.float32
    dt_qkv = getattr(dt, cfg["dt_qkv"])    # x, Wqkv storage (qkv matmul ins)
    dt_qk = getattr(dt, cfg["dt_qk"])      # Q^T/K^T storage (scores matmul ins)
    dt_av = getattr(dt, cfg["dt_av"])      # E, V storage (AV matmul ins)
    dt_proj = getattr(dt, cfg["dt_proj"])  # O^T, Wproj storage (proj matmul ins)
    # matmul-issue dtypes (may be float32r views of float32 storage)
    mm_qkv = getattr(dt, cfg["mm_qkv"])
    mm_qk = getattr(dt, cfg["mm_qk"])
    mm_av = getattr(dt, cfg["mm_av"])
    mm_proj = getattr(dt, cfg["mm_proj"])

    nc = bacc.Bacc("TRN2", target_bir_lowering=False, debug=False,
                   num_devices=N_CORES)

    xT_d = nc.dram_tensor("xT", [C, N], dt_qkv, kind="ExternalInput")
    wqkvT_d = nc.dram_tensor("wqkvT", [C, 3 * C], dt_qkv, kind="ExternalInput")
    wprojT_d = nc.dram_tensor("wprojT", [C, C], dt_proj, kind="ExternalInput")
    bqk_d = nc.dram_tensor("bqk", [P, 2 * C // P], f32, kind="ExternalInput")
    bp_d = nc.dram_tensor("bp", [1, C], f32, kind="ExternalInput")
    y_d = nc.dram_tensor("y", [N, C], f32, kind="ExternalOutput")

    def mmv(ap, mmdt):
        # reinterpret a float32 AP as float32r for fast matmul issue
        return ap.bitcast(mmdt) if mmdt != ap.dtype else ap

    with tile.TileContext(nc, pool_alloc_mode="queue") as tc:
        with (
            tc.tile_pool(name="const", bufs=1) as cpool,
            tc.tile_pool(name="et", bufs=cfg["et_bufs"]) as etpool,
            tc.tile_pool(name="work", bufs=2) as workpool,
            tc.tile_pool(name="ps_s", bufs=2, space="PSUM") as ps_s,
            tc.tile_pool(name="ps_o", bufs=4, space="PSUM") as ps_o,
        ):
            # ---- resident loads ----
            # x + small tensors on the Sync HWDGE queue; the big Wqkv on the
            # Scalar HWDGE queue so the two streams load in parallel.
            xT = [cpool.tile([P, N], dt_qkv, name=f"xT{k}", tag=f"xT{k}") for k in range(KT)]
            for k in range(KT):
                nc.sync.dma_start(xT[k][:], xT_d.ap()[k * P:(k + 1) * P, :])
            # Q/K weight columns on the Scalar queue, V columns on the GpSimd
            # (SWDGE) queue — three DMA streams load in parallel
            wq = [cpool.tile([P, 3 * C], dt_qkv, name=f"wq{k}", tag=f"wq{k}") for k in range(KT)]
            for k in range(KT):
                nc.scalar.dma_start(wq[k][:, 0:2 * C],
                                    wqkvT_d.ap()[k * P:(k + 1) * P, 0:2 * C])
            for k in range(KT):
                nc.gpsimd.dma_start(wq[k][:, 2 * C:3 * C],
                                    wqkvT_d.ap()[k * P:(k + 1) * P, 2 * C:3 * C])
            bqk = cpool.tile([P, 2 * C // P], f32, name="bqk", tag="bqk")
            nc.sync.dma_start(bqk[:], bqk_d.ap())
            bp = cpool.tile([1, C], f32, name="bp", tag="bp")
            nc.sync.dma_start(bp[:], bp_d.ap())
            bp_b = cpool.tile([P, C], f32, name="bp_b", tag="bp_b")
            nc.gpsimd.partition_broadcast(bp_b[:], bp[:])
            wp = [cpool.tile([P, C], dt_proj, name=f"wp{k}", tag=f"wp{k}") for k in range(KT)]
            for k in range(KT):
                nc.sync.dma_start(wp[k][:], wprojT_d.ap()[k * P:(k + 1) * P, :])

            # ---- phase 1: Q^T, K^T in [c', n] tiles ----
            # Only the first two head-pairs' tiles are emitted up front; the
            # rest are interleaved into the attention stream as PE filler
            # (attention is ACT/exp-bound, PE has idle slots), two pairs
            # ahead of their consumer so the scheduler has slack.
            qkT = [cpool.tile([P, N], dt_qk, name=f"qkT{t}", tag=f"qkT{t}")
                   for t in range(2 * C // P)]

            def qk_group(t, g, pool, tag):
                pm = pool.tile([P, 512], f32, name="mm", tag=tag)
                for k in range(KT):
                    nc.tensor.matmul(
                        pm[:],
                        mmv(wq[k][:, t * P:(t + 1) * P], mm_qkv),
                        mmv(xT[k][:, g * 512:(g + 1) * 512], mm_qkv),
                        start=(k == 0), stop=(k == KT - 1),
                    )
                nc.vector.tensor_scalar_add(
                    qkT[t][:, g * 512:(g + 1) * 512], pm[:],
                    bqk[:, t:t + 1])

            NQT = 2 * C // P  # 12 q/k tiles; pair p uses tiles p and 6+p

            # pair 0 up front — unblocks attention immediately
            for t in (0, NQT // 2):
                for g in range(2):
                    qk_group(t, g, ps_o, "o")

            # ---- phase 2: V token-major with ones columns; emitted as
            # filler units inside h0's slots (h0 has no AV work yet)
            v = [cpool.tile([P, H * HD], dt_av, name=f"v{nt}", tag=f"v{nt}")
                 for nt in range(NT)]

            def v_unit(nt):
                nc.vector.memset(
                    v[nt][:].rearrange("p (h d) -> p h d", d=HD)[:, :, D:HD],
                    1.0)
                dst = v[nt][:].rearrange("p (h d) -> p h d", d=HD)
                for off, width in ((0, 512), (512, 256)):
                    pm = ps_o.tile([P, 512], f32, name="mm", tag="o")
                    for k in range(KT):
                        nc.tensor.matmul(
                            pm[:, 0:width],
                            mmv(xT[k][:, nt * P:(nt + 1) * P], mm_qkv),
                            mmv(wq[k][:, 2 * C + off:2 * C + off + width],
                                mm_qkv),
                            start=(k == 0), stop=(k == KT - 1),
                        )
                    nh, h0_ = width // D, off // D
                    src = pm[:, 0:width].rearrange("p (h d) -> p h d", d=D)
                    nc.vector.tensor_copy(dst[:, h0_:h0_ + nh, 0:D], src[:])

            ouT = [cpool.tile([P, N], dt_proj, name=f"ouT{j}", tag=f"ouT{j}")
                   for j in range(KT)]
            ysb = [workpool.tile([P, C], f32, name=f"ysb{nt}", tag=f"ysb{nt}",
                                 bufs=1) for nt in range(NT)]

            # Filler schedule, one unit per (head, m-tile) slot. V(nt) pops
            # at h0's slot nt (its consumer av(h0, nt) runs a full head
            # later). Pair p's q/k groups pop at head p, slots 4-7 (away
            # from head-boundary PSUM pressure), ready before head 2p.
            slot_fill: dict[tuple[int, int], object] = {}
            for nt in range(NT):
                slot_fill[(0, nt)] = (lambda nt=nt: v_unit(nt))
            for p in range(1, NQT // 2):
                units = [(lambda t=t, g=g: qk_group(t, g, ps_o, "o"))
                         for t in (p, NQT // 2 + p) for g in range(2)]
                for i, u in enumerate(units):
                    slot_fill[(p, 4 + i)] = u

            class HeadState:
                def __init__(self, h):
                    self.h = h
                    self.off = D * (h % 2)
                    self.ets = []
                    self.o_ps = None

            def score_exp(st, mt):
                qt = qkT[st.h // 2]
                kt = qkT[NQT // 2 + st.h // 2]
                sp = ps_s.tile([P, N], f32, name="sp", tag="s")
                for g in range(2):
                    nc.tensor.matmul(
                        sp[:, g * 512:(g + 1) * 512],
                        mmv(kt[st.off:st.off + D, mt * P:(mt + 1) * P], mm_qk),
                        mmv(qt[st.off:st.off + D,
                               g * 512:(g + 1) * 512], mm_qk),
                        start=True, stop=True,
                    )
                et = etpool.tile([P, N], dt_av, name="et", tag="et")
                nc.scalar.activation(
                    et[:], sp[:],
                    bass.mybir.ActivationFunctionType.Exp,
                    scale=float(1.0 / np.sqrt(D)))
                st.ets.append(et)

            def av(st, mt):
                if st.o_ps is None:
                    st.o_ps = [ps_o.tile([HD, 512], f32, name="o_ps", tag="o")
                               for _ in range(2)]
                for g in range(2):
                    nc.tensor.matmul(
                        st.o_ps[g][:],
                        mmv(v[mt][:, st.h * HD:(st.h + 1) * HD], mm_av),
                        mmv(st.ets[mt][:, g * 512:(g + 1) * 512], mm_av),
                        start=(mt == 0), stop=(mt == NT - 1),
                    )

            def normalize(st):
                # one independent chain per 512-half so PSUM slots free as
                # early as possible. The sum row is staged via SBUF: the
                # custom-DVE reciprocal mis-reads PSUM at base partition 64
                # on HW (sim is fine).
                s_sb = workpool.tile([1, N], f32, name="s_sb", tag="s_sb")
                r = workpool.tile([1, N], f32, name="r", tag="r")
                rb = workpool.tile([D, N], f32, name="rb", tag="rb")
                for g in range(2):
                    sl = slice(g * 512, (g + 1) * 512)
                    nc.vector.tensor_copy(s_sb[0:1, sl], st.o_ps[g][D:HD, :])
                    nc.vector.reciprocal_approx_fast(r[0:1, sl],
                                                     s_sb[0:1, sl])
                    nc.gpsimd.partition_broadcast(rb[:, sl], r[0:1, sl])
                    nc.vector.tensor_mul(
                        ouT[st.h // 2][st.off:st.off + D, sl],
                        st.o_ps[g][0:D, :], rb[:, sl])

            # Heads are software-pipelined one full head deep: head h's slots
            # run its scores/exp plus head h-1's AV matmuls, so the in-order
            # PE queue never waits on the exp latency and head h-1's PSUM
            # tail never blocks head h's scores.
            prev = None
            for h in range(H):
                st = HeadState(h)
                for mt in range(NT):
                    score_exp(st, mt)
                    if prev is not None:
                        av(prev, mt)
                    u = slot_fill.pop((h, mt), None)
                    if u is not None:
                        u()
                if prev is not None:
                    normalize(prev)
                prev = st
            for mt in range(NT):
                av(prev, mt)
            normalize(prev)

            # ---- phase 4: proj; bias via broadcast add fused into the
            # PSUM->SBUF move. Uses the scores PSUM slots (idle by now) so
            # consecutive n-tiles double-buffer.
            for nt in range(NT):
                pm = ps_s.tile([P, N], f32, name="mm", tag="s")
                for off, width in ((0, 512), (512, 256)):
                    for k in range(KT):
                        nc.tensor.matmul(
                            pm[:, off:off + width],
                            mmv(ouT[k][:, nt * P:(nt + 1) * P], mm_proj),
                            mmv(wp[k][:, off:off + width], mm_proj),
                            start=(k == 0), stop=(k == KT - 1),
                        )
                nc.vector.tensor_add(ysb[nt][:], pm[:, 0:C], bp_b[:])
                nc.sync.dma_start(y_d.ap()[nt * P:(nt + 1) * P, :], ysb[nt][:])

    nc.compile()
    return nc


DEFAULT_CFG = dict(
    dt_qkv="bfloat16", dt_qk="bfloat16", dt_av="bfloat16", dt_proj="bfloat16",
    mm_qkv="bfloat16", mm_qk="bfloat16", mm_av="bfloat16", mm_proj="bfloat16",
    et_bufs=12,
)


def _np_dt(name):
    return {"bfloat16": ml_dtypes.bfloat16, "float32": np.float32}[name]


def _host_prep(x, W_qkv, b_qkv, W_proj, b_proj, cfg):
    """Shard + lay out host-side numpy inputs per core."""
    dqkv = _np_dt(cfg["dt_qkv"])
    dproj = _np_dt(cfg["dt_proj"])
    wqkvT = np.ascontiguousarray(W_qkv.T).astype(dqkv)
    wprojT = np.ascontiguousarray(W_proj.T).astype(dproj)
    bqk = np.ascontiguousarray(
        b_qkv[:2 * C].reshape(2 * C // P, P).T).astype(np.float32)
    bp_eff = (b_proj.astype(np.float64)
              + W_proj.astype(np.float64) @ b_qkv[2 * C:].astype(np.float64))
    bp = bp_eff.astype(np.float32).reshape(1, C)
    in_maps = []
    for b in range(N_CORES):
        xT = np.ascontiguousarray(x[b].T).astype(dqkv)
        in_maps.append({"xT": xT, "wqkvT": wqkvT, "wprojT": wprojT,
                        "bqk": bqk, "bp": bp})
    return in_maps


def get_nc(cfg=None):
    cfg = dict(DEFAULT_CFG, **(cfg or {}))
    key = tuple(sorted(cfg.items()))
    if key not in _CACHE:
        _CACHE[key] = _build(cfg)
    return _CACHE[key]


def run(inputs, cfg=None, **run_kwargs):
    from concourse import bass_utils

    cfg = dict(DEFAULT_CFG, **(cfg or {}))
    nc = get_nc(cfg)
    in_maps = _host_prep(inputs["x"], inputs["W_qkv"], inputs["b_qkv"],
                         inputs["W_proj"], inputs["b_proj"], cfg)
    res = bass_utils.run_bass_kernel_spmd(
        nc, in_maps, core_ids=list(range(N_CORES)), **run_kwargs)
    out = np.stack([res.results[b]["y"] for b in range(N_CORES)], axis=0)
    return out, res


def kernel(**inputs) -> np.ndarray:
    inputs = {k: np.asarray(v) for k, v in inputs.items()}
    out, _ = run(inputs)
    return out
